# revision 33
# baseline (speedup 1.0000x reference)
"""GQA attention Trainium2 kernel (8 NeuronCores, SPMD, no collectives).

Sharding: 2-way data parallel (batch) x 4-way tensor parallel (heads).
Core c handles batch b=c//4 and head-group g=c%4 (8 q heads, 2 kv heads).
Each core produces a partial o_proj output (transposed, [HID, S] f32);
the host sums the 4 partials per batch and transposes back.

v2: fully software-pipelined over 512-position sequence blocks so the PE
never idles long enough to re-throttle (HAM), all transcendentals come
from ONE activation table set (natural_log_exp_and_others: Exp/Ln/Square/
Copy; sigmoid = exp(-ln(1+exp(-x))), rstd = exp(-0.5*ln(mean_sq)),
1/denom = exp(-ln(denom))), rope rotation is a small permutation matmul,
V is projected directly into natural [keys, dims] layout (h-block as
stationary operand), and score matmuls for the two kv-head groups run
concurrently in disjoint PE row groups. Heads are reordered host-side as
(m, m+4) pairs so every per-chunk elementwise op is a single full
[128,512] instruction.
"""

import os
import sys
import numpy as np

for _p in ("/opt/trn_rl_repo", "/root/.axon_site/_ro/trn_rl_repo"):
    if os.path.isdir(_p) and _p not in sys.path:
        sys.path.insert(0, _p)

import ml_dtypes

B, S, HID = 2, 2048, 2048
NH, NKV, HD = 32, 8, 64
ROPE = 32
EPS = 1e-6
SCALE = HD ** -0.5
NCORES = 8
QH = NH // 4      # 8 q heads per core
KVH = NKV // 4    # 2 kv heads per core
QD = QH * HD      # 512 per-core q dim
KD = KVH * HD     # 128 per-core kv dim
KC = HID // 128   # 16 contraction chunks
SB = S // 512     # 4 sequence blocks of 512
BF16 = ml_dtypes.bfloat16

_CACHE = {}


def _steer_act_tables_to_one_set():
    """All ACT funcs used here (Exp/Ln/Square/Copy/Identity) exist in the
    'natural_log_exp_and_others' table set, but bacc's load-insertion pass
    picks the first set containing each func, thrashing between set 0 (exp)
    and set 5 (ln) with a ~2.7us ACT_TABLE_LOAD at every transition. Strip
    those funcs from every other set in the (cached) table map so the pass
    resolves them all to the single covering set. Names and order are
    preserved, so act_func_set_id stays valid and the loaded table really
    does contain every function we run.
    """
    import functools
    from concourse import hw_specs, bacc, mybir

    if getattr(_steer_act_tables_to_one_set, "_done", False):
        return
    orig = hw_specs.get_activation_tables

    @functools.cache
    def patched(arch):
        t = orig(arch)
        target = "natural_log_exp_and_others"
        if target not in t:
            return t
        strip = t[target]
        return {name: (fns if name == target else fns - strip)
                for name, fns in t.items()}

    hw_specs.get_activation_tables = patched
    if getattr(bacc, "get_activation_tables", None) is orig:
        bacc.get_activation_tables = patched
    _steer_act_tables_to_one_set._done = True


def _build_bass(debug_dump=False):
    import concourse.bass as bass
    from concourse import bacc, mybir, tile

    _steer_act_tables_to_one_set()

    f32 = mybir.dt.float32
    bf16 = mybir.dt.bfloat16

    nc = bacc.Bacc("TRN2", target_bir_lowering=False, debug=False,
                   enable_asserts=False, num_devices=NCORES)

    hT = nc.dram_tensor("hT", [HID, S], bf16, kind="ExternalInput").ap()
    wqT = nc.dram_tensor("wqT", [HID, QD], bf16, kind="ExternalInput").ap()
    wkT = nc.dram_tensor("wkT", [HID, KD], bf16, kind="ExternalInput").ap()
    wvT = nc.dram_tensor("wvT", [HID, KD], bf16, kind="ExternalInput").ap()
    wgT = nc.dram_tensor("wgT", [HID, QD], bf16, kind="ExternalInput").ap()
    woT = nc.dram_tensor("woT", [QD, HID], bf16, kind="ExternalInput").ap()
    csAq = nc.dram_tensor("csAq", [128, S], bf16, kind="ExternalInput").ap()
    csBq = nc.dram_tensor("csBq", [128, S], bf16, kind="ExternalInput").ap()
    csAk = nc.dram_tensor("csAk", [128, S], bf16, kind="ExternalInput").ap()
    csBk = nc.dram_tensor("csBk", [128, S], bf16, kind="ExternalInput").ap()
    rotP = nc.dram_tensor("rotP", [128, 128], bf16, kind="ExternalInput").ap()
    outT = nc.dram_tensor("outT", [HID, S], bf16, kind="ExternalOutput").ap()
    if debug_dump:
        dbg_q = nc.dram_tensor("dbg_q", [128, 4, S], bf16,
                               kind="ExternalOutput").ap()
        dbg_k = nc.dram_tensor("dbg_k", [128, S], bf16,
                               kind="ExternalOutput").ap()
        dbg_g = nc.dram_tensor("dbg_g", [128, 4, S], bf16,
                               kind="ExternalOutput").ap()
        dbg_og = nc.dram_tensor("dbg_og", [128, 4, S], bf16,
                                kind="ExternalOutput").ap()
        dbg_v = nc.dram_tensor("dbg_v", [128, KC, KVH, 128], bf16,
                               kind="ExternalOutput").ap()
        dbg_pr = nc.dram_tensor("dbg_pr", [128, SB, KC, 2, 512], bf16,
                                kind="ExternalOutput").ap()
        dbg_rb = nc.dram_tensor("dbg_rb", [128, SB, 512], f32,
                                kind="ExternalOutput").ap()
        dbg_ln = nc.dram_tensor("dbg_ln", [128, SB, 512], f32,
                                kind="ExternalOutput").ap()
        dbg_rd = nc.dram_tensor("dbg_rd", [128, SB, 512], f32,
                                kind="ExternalOutput").ap()
        dbg_av = nc.dram_tensor("dbg_av", [128, SB, 512], f32,
                                kind="ExternalOutput").ap()

    Exp = mybir.ActivationFunctionType.Exp
    Ln = mybir.ActivationFunctionType.Ln
    Square = mybir.ActivationFunctionType.Square
    PSUM = bass.MemorySpace.PSUM

    with tile.TileContext(nc) as tc:
        with tc.tile_pool(name="persist", bufs=1) as pp, \
             tc.tile_pool(name="rot2", bufs=3) as r2, \
             tc.tile_pool(name="rot2b", bufs=2) as r2b, \
             tc.tile_pool(name="hpool", bufs=3) as hp, \
             tc.tile_pool(name="work", bufs=2) as wk, \
             tc.tile_pool(name="attw", bufs=4) as aw, \
             tc.tile_pool(name="ogw", bufs=2) as ow, \
             tc.tile_pool(name="ostg", bufs=2) as ostg, \
             tc.tile_pool(name="ps_big", bufs=1, space=PSUM) as psb, \
             tc.tile_pool(name="ps_aux", bufs=1, space=PSUM) as psx, \
             tc.tile_pool(name="ps_sc", bufs=1, space=PSUM) as pssc, \
             tc.tile_pool(name="ps_av", bufs=1, space=PSUM) as psav:

            # ---------------- persistent sbuf ----------------
            kT_sb = pp.tile([128, S], bf16)           # normed+roped k
            v_sb = pp.tile([128, KC, KVH, 128], bf16)  # natural V + ones col
            wq_sb = pp.tile([128, KC, QD], bf16)
            wk_sb = pp.tile([128, KC, KD], bf16)
            wv_sb = pp.tile([128, KC, KD], bf16)
            wg_sb = pp.tile([128, KC, QD], bf16)
            wo_sb = pp.tile([128, 4, KC, 128], bf16)
            csA_q = pp.tile([128, S], bf16)
            csB_q = pp.tile([128, S], bf16)
            csA_k = pp.tile([128, S], bf16)
            csB_k = pp.tile([128, S], bf16)
            rot_sb = pp.tile([128, 128], bf16)
            ones128 = pp.tile([128, 128], bf16)
            eps_t = pp.tile([128, 1], f32)
            nc.vector.memset(eps_t, EPS)

            # v_sb: [., kc, 0, 0:64]=V_kv0, [., kc, 0, 64]=1 (denom col A)
            #       [., kc, 1, 64:128]=V_kv1, [., kc, 1, 0]=1 (denom col B)
            nc.vector.memset(v_sb, 0.0)
            nc.vector.memset(v_sb[:, :, 0, 64:65], 1.0)
            nc.vector.memset(v_sb[:, :, 1, 0:1], 1.0)
            # block-diag ones for per-head-half squared sums
            nc.vector.memset(ones128, 0.0)
            nc.vector.memset(ones128[0:64, 0:64], 1.0)
            nc.vector.memset(ones128[64:128, 64:128], 1.0)

            # ---- input DMAs, split so early consumers start early ----
            nc.sync.dma_start(out=wk_sb,
                              in_=wkT.rearrange("(c p) m -> p c m", p=128))
            nc.sync.dma_start(out=wv_sb,
                              in_=wvT.rearrange("(c p) m -> p c m", p=128))
            wq_r = wqT.rearrange("(c p) m -> p c m", p=128)
            wg_r = wgT.rearrange("(c p) m -> p c m", p=128)
            for m in range(4):
                nc.sync.dma_start(out=wq_sb[:, :, m * 128:(m + 1) * 128],
                                  in_=wq_r[:, :, m * 128:(m + 1) * 128])
            nc.sync.dma_start(out=rot_sb, in_=rotP)
            for sb in range(SB):
                sl = slice(sb * 512, (sb + 1) * 512)
                nc.sync.dma_start(out=csA_k[:, sl], in_=csAk[:, sl])
                nc.sync.dma_start(out=csB_k[:, sl], in_=csBk[:, sl])
                nc.sync.dma_start(out=csA_q[:, sl], in_=csAq[:, sl])
                nc.sync.dma_start(out=csB_q[:, sl], in_=csBq[:, sl])
            for m in range(4):
                nc.sync.dma_start(out=wg_sb[:, :, m * 128:(m + 1) * 128],
                                  in_=wg_r[:, :, m * 128:(m + 1) * 128])
            nc.sync.dma_start(out=wo_sb,
                              in_=woT.rearrange("(c p) (mb mm) -> p c mb mm",
                                                p=128, mm=128))

            # rotating per-block tensors
            def qT_tile():
                return r2.tile([128, 4, 512], bf16, tag="qT", name="qT")

            def g_tile():
                return r2.tile([128, 4, 512], bf16, tag="gT", name="gT")

            def og_tile():
                return r2b.tile([128, 4, 512], bf16, tag="ogT", name="ogT")

            qT_blk = [None] * SB
            g_blk = [None] * SB
            og_blk = [None] * SB

            def load_hblk(sb):
                """h-block as two [128, 8, 512] half-tiles (kc 0:8, 8:16)."""
                src = hT[:, sb * 512:(sb + 1) * 512]
                r = src.rearrange("(c p) s -> p c s", p=128)
                ta = hp.tile([128, 8, 512], bf16, tag="hbA", name="hbA")
                tb = hp.tile([128, 8, 512], bf16, tag="hbB", name="hbB")
                nc.sync.dma_start(out=ta, in_=r[:, 0:8, :])
                nc.sync.dma_start(out=tb, in_=r[:, 8:16, :])

                class HB:
                    def __getitem__(self, idx):
                        p, kc, s = idx
                        return (ta[p, kc, s] if kc < 8
                                else tb[p, kc - 8, s])
                return HB()

            def rope_norm_chunk(hblk, w_sb, mslice, csA, csB, out_dst):
                """Project one 128-row chunk, rope+rms-norm it, write bf16."""
                ps = psb.tile([128, 512], f32, tag="big")
                for kc in range(KC):
                    nc.tensor.matmul(ps, w_sb[:, kc, mslice], hblk[:, kc, :],
                                     start=(kc == 0), stop=(kc == KC - 1))
                # single fast read of ps, then everything works off qsb
                qsb = wk.tile([128, 512], bf16, tag="qsb")
                nc.vector.tensor_copy(out=qsb, in_=ps)
                # squared sums per 64-row half -> replicated across the half
                sq_t = wk.tile([128, 512], bf16, tag="sq")
                nc.scalar.activation(out=sq_t, in_=qsb, func=Square)
                sqps = psx.tile([128, 512], f32, tag="aux")
                nc.tensor.matmul(sqps, ones128, sq_t, start=True, stop=True)
                # rstd = exp(-0.5*ln(ms/HD + eps))
                lnm = wk.tile([128, 512], f32, tag="lnm")
                nc.scalar.activation(out=lnm, in_=sqps, func=Ln,
                                     scale=1.0 / HD, bias=eps_t)
                rb = wk.tile([128, 512], f32, tag="rb")
                nc.scalar.activation(out=rb, in_=lnm, func=Exp, scale=-0.5)
                # rope: qa = qsb*csA + (rotP @ qsb)*csB
                rps = psx.tile([128, 512], f32, tag="aux")
                nc.tensor.matmul(rps, rot_sb, qsb, start=True, stop=True)
                a1 = wk.tile([128, 512], f32, tag="a1")
                nc.vector.tensor_mul(a1, qsb, csA)
                a2 = wk.tile([128, 512], f32, tag="a2")
                nc.vector.tensor_mul(a2, rps, csB)
                nc.vector.tensor_add(a1, a1, a2)
                nc.vector.tensor_mul(out_dst, a1, rb)

            def proj_block(sb):
                s0 = sb * 512
                sl = slice(s0, s0 + 512)
                hblk = load_hblk(sb)
                # ---- k (both kv heads in one chunk) ----
                rope_norm_chunk(hblk, wk_sb, slice(0, KD),
                                csA_k[:, sl], csB_k[:, sl], kT_sb[:, sl])
                # ---- v in natural layout: h-block chunk as stationary ----
                for cc in range(4):
                    vps = psx.tile([128, 128], f32, tag="aux")
                    for kc in range(KC):
                        nc.tensor.matmul(vps,
                                         hblk[:, kc, cc * 128:(cc + 1) * 128],
                                         wv_sb[:, kc, :],
                                         start=(kc == 0), stop=(kc == KC - 1))
                    chunk = sb * 4 + cc
                    nc.vector.tensor_copy(out=v_sb[:, chunk, 0, 0:64],
                                          in_=vps[:, 0:64])
                    nc.vector.tensor_copy(out=v_sb[:, chunk, 1, 64:128],
                                          in_=vps[:, 64:128])
                # ---- q (4 chunks; chunk m = heads (m, m+4)) ----
                qT = qT_tile()
                qT_blk[sb] = qT
                for m in range(4):
                    rope_norm_chunk(hblk, wq_sb,
                                    slice(m * 128, (m + 1) * 128),
                                    csA_q[:, sl], csB_q[:, sl], qT[:, m, :])
                # ---- gate: sigmoid(x) = exp(-ln(1+exp(-x))) ----
                g = g_tile()
                g_blk[sb] = g
                for m in range(4):
                    ps = psb.tile([128, 512], f32, tag="big")
                    for kc in range(KC):
                        nc.tensor.matmul(ps, wg_sb[:, kc,
                                                   m * 128:(m + 1) * 128],
                                         hblk[:, kc, :],
                                         start=(kc == 0), stop=(kc == KC - 1))
                    e = wk.tile([128, 512], f32, tag="a1", name="ge")
                    nc.scalar.activation(out=e, in_=ps, func=Exp, scale=-1.0)
                    t = wk.tile([128, 512], f32, tag="a2", name="gt")
                    nc.scalar.activation(out=t, in_=e, func=Ln, bias=1.0)
                    nc.scalar.activation(out=g[:, m, :], in_=t, func=Exp,
                                         scale=-1.0)

            def attn_block(sb):
                """Attention for q block sb (512 queries), all 4 head pairs."""
                s0 = sb * 512
                nkc = 4 * (sb + 1)
                qT = qT_blk[sb]
                g = g_blk[sb]
                og = og_tile()
                og_blk[sb] = og
                for p in range(4):
                    av = psav.tile([128, 2, 512], f32, tag="av")
                    avA = av[:, 0, :]
                    avB = av[:, 1, :]
                    scq = pssc.tile([128, 2, 2, 512], f32, tag="sc")
                    for kc in range(nkc):
                        rr = kc % 2
                        diag_i = kc - 4 * sb
                        trim = 128 * diag_i if diag_i > 0 else 0
                        n = 512 - trim
                        qs = slice(trim, 512)
                        sc = scq[:, rr, :, :]
                        nc.tensor.matmul(
                            sc[:, 0, qs],
                            kT_sb[0:64, kc * 128:(kc + 1) * 128],
                            qT[0:64, p, qs], start=True, stop=True)
                        nc.tensor.matmul(
                            sc[:, 1, qs],
                            kT_sb[64:128, kc * 128:(kc + 1) * 128],
                            qT[64:128, p, qs], start=True, stop=True)
                        probs = aw.tile([128, 2, 512], bf16, tag="probs")
                        nc.scalar.activation(
                            out=probs[:, :, qs], in_=sc[:, :, qs],
                            func=Exp, scale=SCALE)
                        if diag_i >= 0:
                            nc.gpsimd.affine_select(
                                out=probs[:, :, qs], in_=probs[:, :, qs],
                                compare_op=mybir.AluOpType.is_ge,
                                fill=0.0,
                                base=s0 + trim - kc * 128,
                                channel_multiplier=-1,
                                pattern=[[0, 2], [1, n]])
                        if debug_dump and p == 0:
                            nc.sync.dma_start(out=dbg_pr[:, sb, kc, :, qs],
                                              in_=probs[:, :, qs])
                        nc.tensor.matmul(
                            av[0:65, 0, qs], v_sb[:, kc, 0, 0:65],
                            probs[:, 0, qs],
                            start=(kc == 0), stop=(kc == nkc - 1))
                        nc.tensor.matmul(
                            av[:, 1, qs], v_sb[:, kc, 1, :],
                            probs[:, 1, qs],
                            start=(kc == 0), stop=(kc == nkc - 1))
                    # normalize + gate. A: dims rows 0:64, denom row 64.
                    #                   B: dims rows 64:128, denom row 0.
                    lnA = ow.tile([128, 512], f32, tag="lnd")
                    nc.scalar.activation(out=lnA[64:65, :], in_=av[64:65, 0, :],
                                         func=Ln)
                    nc.scalar.activation(out=lnA[0:1, :], in_=av[0:1, 1, :],
                                         func=Ln)
                    if debug_dump and p == 0:
                        avd = ow.tile([128, 512], f32, tag="avd", name="avd")
                        nc.vector.tensor_copy(out=avd[64:65, :],
                                              in_=av[64:65, 0, :])
                        nc.vector.tensor_copy(out=avd[0:1, :],
                                              in_=av[0:1, 1, :])
                        nc.sync.dma_start(out=dbg_av[:, sb, :], in_=avd)
                        nc.sync.dma_start(out=dbg_ln[:, sb, :], in_=lnA)
                    rA = ow.tile([128, 512], f32, tag="rd")
                    nc.scalar.activation(out=rA[64:65, :], in_=lnA[64:65, :],
                                         func=Exp, scale=-1.0)
                    nc.scalar.activation(out=rA[0:1, :], in_=lnA[0:1, :],
                                         func=Exp, scale=-1.0)
                    # partition_broadcast only works base0 -> base0; route
                    # cross-partition moves through (gpsimd) DMA instead.
                    rbv = ow.tile([128, 512], f32, tag="lnd", name="rbv")
                    rdup = ow.tile([128, 512], f32, tag="rdup")
                    rbt = ow.tile([64, 512], f32, tag="rdup", name="rbt")
                    nc.gpsimd.dma_start(out=rdup[0:1, :], in_=rA[64:65, :])
                    nc.gpsimd.partition_broadcast(rbv[0:64, :], rdup[0:1, :])
                    nc.gpsimd.partition_broadcast(rbt[0:64, :], rA[0:1, :])
                    nc.gpsimd.dma_start(out=rbv[64:128, :], in_=rbt[0:64, :])
                    if debug_dump and p == 0:
                        nc.sync.dma_start(out=dbg_rb[:, sb, :], in_=rbv)
                        nc.sync.dma_start(out=dbg_rd[:, sb, :], in_=rA)
                    tmpA = ow.tile([128, 512], bf16, tag="lnd", name="ogt")
                    nc.vector.tensor_mul(tmpA[0:64, :], av[0:64, 0, :],
                                         rbv[0:64, :])
                    nc.vector.tensor_mul(tmpA[64:128, :], av[64:128, 1, :],
                                         rbv[64:128, :])
                    nc.vector.tensor_mul(og[0:64, p, :], tmpA[0:64, :],
                                         g[0:64, p, :])
                    nc.vector.tensor_mul(og[64:128, p, :], tmpA[64:128, :],
                                         g[64:128, p, :])

            def oproj_block(sb):
                og = og_blk[sb]
                for m in range(KC):
                    po = psb.tile([128, 512], f32, tag="big")
                    for oc in range(4):
                        nc.tensor.matmul(po, wo_sb[:, oc, m, :],
                                         og[:, oc, :],
                                         start=(oc == 0), stop=(oc == 3))
                    stg = ostg.tile([128, 512], bf16, tag="stg")
                    nc.vector.tensor_copy(out=stg, in_=po)
                    nc.sync.dma_start(
                        out=outT[m * 128:(m + 1) * 128,
                                 sb * 512:(sb + 1) * 512],
                        in_=stg)

            for sb in range(SB):
                proj_block(sb)
                attn_block(sb)
                oproj_block(sb)
                if debug_dump:
                    ssl = slice(sb * 512, (sb + 1) * 512)
                    nc.sync.dma_start(out=dbg_q[:, :, ssl], in_=qT_blk[sb])
                    nc.sync.dma_start(out=dbg_g[:, :, ssl], in_=g_blk[sb])
                    nc.sync.dma_start(out=dbg_og[:, :, ssl], in_=og_blk[sb])
            if debug_dump:
                nc.sync.dma_start(out=dbg_k, in_=kT_sb)
                nc.sync.dma_start(out=dbg_v, in_=v_sb)

    nc.compile()
    return nc


def _host_prep(hidden_states, cos, sin, Wq, Wk, Wv, Wg, Wo, q_norm_w, k_norm_w):
    """Build per-core input maps."""
    half = ROPE // 2  # 16
    # head order within a core: pairs (m, m+4) -> chunk m
    perm = [0, 4, 1, 5, 2, 6, 3, 7]

    def cs_tables(cos_b, sin_b, w):
        # csA/csB [128, S]: row p -> head-local dim d = p % 64
        A = np.empty((128, S), np.float32)
        Bt = np.empty((128, S), np.float32)
        cosT = cos_b.T  # [32, S]
        sinT = sin_b.T
        for blk in (0, 64):
            A[blk + 0:blk + 32] = cosT * w[0:32, None]
            A[blk + 32:blk + 64] = w[32:64, None]
            Bt[blk + 0:blk + 16] = -sinT[0:16] * w[16:32, None]
            Bt[blk + 16:blk + 32] = sinT[16:32] * w[0:16, None]
            Bt[blk + 32:blk + 64] = 0.0
        return A.astype(BF16), Bt.astype(BF16)

    # rotP: out[d] = q[d+16] (d%64 in 0:16), q[d-16] (d%64 in 16:32), else 0
    rotP = np.zeros((128, 128), np.float32)
    for hh in (0, 64):
        for j in range(16):
            rotP[hh + 16 + j, hh + j] = 1.0      # out[hh+j] = in[hh+16+j]
            rotP[hh + j, hh + 16 + j] = 1.0      # out[hh+16+j] = in[hh+j]
    rotP = rotP.astype(BF16)

    in_maps = []
    for c in range(NCORES):
        b, g = c // 4, c % 4
        qs = slice(g * QD, (g + 1) * QD)
        ks = slice(g * KD, (g + 1) * KD)
        csA_q, csB_q = cs_tables(cos[b], sin[b], np.asarray(q_norm_w))
        csA_k, csB_k = cs_tables(cos[b], sin[b], np.asarray(k_norm_w))
        Wq_l = Wq[qs].reshape(QH, HD, HID)[perm].reshape(QD, HID)
        Wg_l = Wg[qs].reshape(QH, HD, HID)[perm].reshape(QD, HID)
        Wo_l = Wo[:, qs].reshape(HID, QH, HD)[:, perm].reshape(HID, QD)
        in_maps.append({
            "hT": np.ascontiguousarray(hidden_states[b].T).astype(BF16),
            "wqT": np.ascontiguousarray(Wq_l.T).astype(BF16),
            "wkT": np.ascontiguousarray(Wk[ks].T).astype(BF16),
            "wvT": np.ascontiguousarray(Wv[ks].T).astype(BF16),
            "wgT": np.ascontiguousarray(Wg_l.T).astype(BF16),
            "woT": np.ascontiguousarray(Wo_l.T).astype(BF16),
            "csAq": csA_q, "csBq": csB_q, "csAk": csA_k, "csBk": csB_k,
            "rotP": rotP,
        })
    return in_maps


def kernel(hidden_states, cos, sin, Wq, Wk, Wv, Wg, Wo, q_norm_w, k_norm_w):
    from concourse import bass_utils

    if "nc" not in _CACHE:
        _CACHE["nc"] = _build_bass()
    nc = _CACHE["nc"]

    in_maps = _host_prep(hidden_states, cos, sin, Wq, Wk, Wv, Wg, Wo,
                         q_norm_w, k_norm_w)

    trace = bool(int(os.environ.get("KERNEL_TRACE", "0")))
    kwargs = {}
    if trace:
        # the agent image's antenv lacks axon_hooks; recreate it from the
        # boot helper so run_bass_kernel_spmd(trace=True) can NTFF-profile
        try:
            import antenv.axon_hooks  # noqa: F401
        except ImportError:
            import types
            sys.path.insert(0, "/root/.axon_site")
            from trn_agent_boot.trn_boot import _ntff_profile_via_ctypes
            hook = _ntff_profile_via_ctypes("/opt/axon/libaxon_pjrt.so")
            mod = types.ModuleType("antenv.axon_hooks")
            mod.get_axon_ntff_profile_hook = lambda: hook
            sys.modules["antenv.axon_hooks"] = mod
        tmpdir = os.environ.get("KERNEL_TRACE_DIR") or None
        kwargs = dict(trace=True, tmpdir=tmpdir)
    res = bass_utils.run_bass_kernel_spmd(nc, in_maps,
                                          core_ids=list(range(NCORES)),
                                          **kwargs)
    if trace and res.exec_time_ns is not None:
        print(f"HW exec time: {res.exec_time_ns} ns")
        _CACHE["exec_time_ns"] = res.exec_time_ns

    out = np.zeros((B, S, HID), np.float32)
    for c in range(NCORES):
        b = c // 4
        out[b] += res.results[c]["outT"].T.astype(np.float32)
    return out


if __name__ == "__main__":
    rng = np.random.default_rng(0)
    hs = rng.standard_normal((B, S, HID), dtype=np.float32)
    cos = rng.random((B, S, ROPE), dtype=np.float32)
    sin = rng.random((B, S, ROPE), dtype=np.float32)
    out = kernel(hidden_states=hs, cos=cos, sin=sin,
                 Wq=rng.standard_normal((NH * HD, HID), dtype=np.float32) * 0.02,
                 Wk=rng.standard_normal((NKV * HD, HID), dtype=np.float32) * 0.02,
                 Wv=rng.standard_normal((NKV * HD, HID), dtype=np.float32) * 0.02,
                 Wg=rng.standard_normal((NH * HD, HID), dtype=np.float32) * 0.02,
                 Wo=rng.standard_normal((HID, NH * HD), dtype=np.float32) * 0.02,
                 q_norm_w=np.ones(HD, np.float32),
                 k_norm_w=np.ones(HD, np.float32))
    print(out.shape, out.dtype)


# revision 34
# speedup vs baseline: 1.2458x; 1.2458x over previous
"""GQA attention Trainium2 kernel (8 NeuronCores, SPMD, no collectives).

Sharding: 2-way data parallel (batch) x 4-way tensor parallel (heads).
Core c handles batch b=c//4 and head-group g=c%4 (8 q heads, 2 kv heads).
Each core produces a partial o_proj output (transposed, [HID, S] f32);
the host sums the 4 partials per batch and transposes back.

v2: fully software-pipelined over 512-position sequence blocks so the PE
never idles long enough to re-throttle (HAM), all transcendentals come
from ONE activation table set (natural_log_exp_and_others: Exp/Ln/Square/
Copy; sigmoid = exp(-ln(1+exp(-x))), rstd = exp(-0.5*ln(mean_sq)),
1/denom = exp(-ln(denom))), rope rotation is a small permutation matmul,
V is projected directly into natural [keys, dims] layout (h-block as
stationary operand), and score matmuls for the two kv-head groups run
concurrently in disjoint PE row groups. Heads are reordered host-side as
(m, m+4) pairs so every per-chunk elementwise op is a single full
[128,512] instruction.
"""

import os
import sys
import numpy as np

for _p in ("/opt/trn_rl_repo", "/root/.axon_site/_ro/trn_rl_repo"):
    if os.path.isdir(_p) and _p not in sys.path:
        sys.path.insert(0, _p)

import ml_dtypes

B, S, HID = 2, 2048, 2048
NH, NKV, HD = 32, 8, 64
ROPE = 32
EPS = 1e-6
SCALE = HD ** -0.5
NCORES = 8
QH = NH // 4      # 8 q heads per core
KVH = NKV // 4    # 2 kv heads per core
QD = QH * HD      # 512 per-core q dim
KD = KVH * HD     # 128 per-core kv dim
KC = HID // 128   # 16 contraction chunks
SB = S // 512     # 4 sequence blocks of 512
BF16 = ml_dtypes.bfloat16

_CACHE = {}


def _steer_act_tables_to_one_set():
    """All ACT funcs used here (Exp/Ln/Square/Copy/Identity) exist in the
    'natural_log_exp_and_others' table set, but bacc's load-insertion pass
    picks the first set containing each func, thrashing between set 0 (exp)
    and set 5 (ln) with a ~2.7us ACT_TABLE_LOAD at every transition. Strip
    those funcs from every other set in the (cached) table map so the pass
    resolves them all to the single covering set. Names and order are
    preserved, so act_func_set_id stays valid and the loaded table really
    does contain every function we run.
    """
    import functools
    from concourse import hw_specs, bacc, mybir

    if getattr(_steer_act_tables_to_one_set, "_done", False):
        return
    orig = hw_specs.get_activation_tables

    @functools.cache
    def patched(arch):
        t = orig(arch)
        target = "natural_log_exp_and_others"
        if target not in t:
            return t
        strip = t[target]
        return {name: (fns if name == target else fns - strip)
                for name, fns in t.items()}

    hw_specs.get_activation_tables = patched
    if getattr(bacc, "get_activation_tables", None) is orig:
        bacc.get_activation_tables = patched
    _steer_act_tables_to_one_set._done = True


def _build_bass(debug_dump=False):
    import concourse.bass as bass
    from concourse import bacc, mybir, tile

    _steer_act_tables_to_one_set()

    f32 = mybir.dt.float32
    bf16 = mybir.dt.bfloat16

    nc = bacc.Bacc("TRN2", target_bir_lowering=False, debug=False,
                   enable_asserts=False, num_devices=NCORES)

    hT = nc.dram_tensor("hT", [HID, S], bf16, kind="ExternalInput").ap()
    wqT = nc.dram_tensor("wqT", [HID, QD], bf16, kind="ExternalInput").ap()
    wkT = nc.dram_tensor("wkT", [HID, KD], bf16, kind="ExternalInput").ap()
    wvT = nc.dram_tensor("wvT", [HID, KD], bf16, kind="ExternalInput").ap()
    wgT = nc.dram_tensor("wgT", [HID, QD], bf16, kind="ExternalInput").ap()
    woT = nc.dram_tensor("woT", [QD, HID], bf16, kind="ExternalInput").ap()
    csAq = nc.dram_tensor("csAq", [128, S], bf16, kind="ExternalInput").ap()
    csBq = nc.dram_tensor("csBq", [128, S], bf16, kind="ExternalInput").ap()
    csAk = nc.dram_tensor("csAk", [128, S], bf16, kind="ExternalInput").ap()
    csBk = nc.dram_tensor("csBk", [128, S], bf16, kind="ExternalInput").ap()
    rotP = nc.dram_tensor("rotP", [128, 128], bf16, kind="ExternalInput").ap()
    cmsk = nc.dram_tensor("cmsk", [128, 512], bf16, kind="ExternalInput").ap()
    outT = nc.dram_tensor("outT", [HID, S], bf16, kind="ExternalOutput").ap()
    if debug_dump:
        dbg_q = nc.dram_tensor("dbg_q", [128, 4, S], bf16,
                               kind="ExternalOutput").ap()
        dbg_k = nc.dram_tensor("dbg_k", [128, S], bf16,
                               kind="ExternalOutput").ap()
        dbg_g = nc.dram_tensor("dbg_g", [128, 4, S], bf16,
                               kind="ExternalOutput").ap()
        dbg_og = nc.dram_tensor("dbg_og", [128, 4, S], bf16,
                                kind="ExternalOutput").ap()
        dbg_v = nc.dram_tensor("dbg_v", [128, KC, KVH, 128], bf16,
                               kind="ExternalOutput").ap()
        dbg_pr = nc.dram_tensor("dbg_pr", [128, SB, KC, 2, 512], bf16,
                                kind="ExternalOutput").ap()
        dbg_rb = nc.dram_tensor("dbg_rb", [128, SB, 512], f32,
                                kind="ExternalOutput").ap()
        dbg_ln = nc.dram_tensor("dbg_ln", [128, SB, 512], f32,
                                kind="ExternalOutput").ap()
        dbg_rd = nc.dram_tensor("dbg_rd", [128, SB, 512], f32,
                                kind="ExternalOutput").ap()
        dbg_av = nc.dram_tensor("dbg_av", [128, SB, 512], f32,
                                kind="ExternalOutput").ap()

    Exp = mybir.ActivationFunctionType.Exp
    Ln = mybir.ActivationFunctionType.Ln
    Square = mybir.ActivationFunctionType.Square
    PSUM = bass.MemorySpace.PSUM

    with tile.TileContext(nc) as tc:
        with tc.tile_pool(name="persist", bufs=1) as pp, \
             tc.tile_pool(name="rot2", bufs=3) as r2, \
             tc.tile_pool(name="rot2b", bufs=2) as r2b, \
             tc.tile_pool(name="hpool", bufs=3) as hp, \
             tc.tile_pool(name="work", bufs=2) as wk, \
             tc.tile_pool(name="attw", bufs=4) as aw, \
             tc.tile_pool(name="ogw", bufs=2) as ow, \
             tc.tile_pool(name="ostg", bufs=2) as ostg, \
             tc.tile_pool(name="ps_big", bufs=1, space=PSUM) as psb, \
             tc.tile_pool(name="ps_aux", bufs=1, space=PSUM) as psx, \
             tc.tile_pool(name="ps_sc", bufs=1, space=PSUM) as pssc, \
             tc.tile_pool(name="ps_av", bufs=1, space=PSUM) as psav:

            # ---------------- persistent sbuf ----------------
            kT_sb = pp.tile([128, S], bf16)           # normed+roped k
            v_sb = pp.tile([128, KC, KVH, 128], bf16)  # natural V + ones col
            wq_sb = pp.tile([128, KC, QD], bf16)
            wk_sb = pp.tile([128, KC, KD], bf16)
            wv_sb = pp.tile([128, KC, KD], bf16)
            wg_sb = pp.tile([128, KC, QD], bf16)
            wo_sb = pp.tile([128, 4, KC, 128], bf16)
            csA_q = pp.tile([128, S], bf16)
            csB_q = pp.tile([128, S], bf16)
            csA_k = pp.tile([128, S], bf16)
            csB_k = pp.tile([128, S], bf16)
            rot_sb = pp.tile([128, 128], bf16)
            cm_sb = pp.tile([128, 512], bf16)
            ones128 = pp.tile([128, 128], bf16)
            eps_t = pp.tile([128, 1], f32)
            nc.vector.memset(eps_t, EPS)

            # v_sb: [., kc, 0, 0:64]=V_kv0, [., kc, 0, 64]=1 (denom col A)
            #       [., kc, 1, 64:128]=V_kv1, [., kc, 1, 0]=1 (denom col B)
            nc.vector.memset(v_sb, 0.0)
            nc.vector.memset(v_sb[:, :, 0, 64:65], 1.0)
            nc.vector.memset(v_sb[:, :, 1, 0:1], 1.0)
            # block-diag ones for per-head-half squared sums
            nc.vector.memset(ones128, 0.0)
            nc.vector.memset(ones128[0:64, 0:64], 1.0)
            nc.vector.memset(ones128[64:128, 64:128], 1.0)

            # ---- input DMAs, split so early consumers start early ----
            nc.sync.dma_start(out=wk_sb,
                              in_=wkT.rearrange("(c p) m -> p c m", p=128))
            nc.sync.dma_start(out=wv_sb,
                              in_=wvT.rearrange("(c p) m -> p c m", p=128))
            wq_r = wqT.rearrange("(c p) m -> p c m", p=128)
            wg_r = wgT.rearrange("(c p) m -> p c m", p=128)
            for m in range(4):
                nc.sync.dma_start(out=wq_sb[:, :, m * 128:(m + 1) * 128],
                                  in_=wq_r[:, :, m * 128:(m + 1) * 128])
            nc.sync.dma_start(out=rot_sb, in_=rotP)
            nc.sync.dma_start(out=cm_sb, in_=cmsk)
            for sb in range(SB):
                sl = slice(sb * 512, (sb + 1) * 512)
                nc.sync.dma_start(out=csA_k[:, sl], in_=csAk[:, sl])
                nc.sync.dma_start(out=csB_k[:, sl], in_=csBk[:, sl])
                nc.sync.dma_start(out=csA_q[:, sl], in_=csAq[:, sl])
                nc.sync.dma_start(out=csB_q[:, sl], in_=csBq[:, sl])
            for m in range(4):
                nc.sync.dma_start(out=wg_sb[:, :, m * 128:(m + 1) * 128],
                                  in_=wg_r[:, :, m * 128:(m + 1) * 128])
            nc.sync.dma_start(out=wo_sb,
                              in_=woT.rearrange("(c p) (mb mm) -> p c mb mm",
                                                p=128, mm=128))

            # rotating per-block tensors
            def qT_tile():
                return r2.tile([128, 4, 512], bf16, tag="qT", name="qT")

            def g_tile():
                return r2.tile([128, 4, 512], bf16, tag="gT", name="gT")

            def og_tile():
                return r2b.tile([128, 4, 512], bf16, tag="ogT", name="ogT")

            qT_blk = [None] * SB
            g_blk = [None] * SB
            og_blk = [None] * SB

            def load_hblk(sb):
                """h-block as two [128, 8, 512] half-tiles (kc 0:8, 8:16)."""
                src = hT[:, sb * 512:(sb + 1) * 512]
                r = src.rearrange("(c p) s -> p c s", p=128)
                ta = hp.tile([128, 8, 512], bf16, tag="hbA", name="hbA")
                tb = hp.tile([128, 8, 512], bf16, tag="hbB", name="hbB")
                nc.sync.dma_start(out=ta, in_=r[:, 0:8, :])
                nc.sync.dma_start(out=tb, in_=r[:, 8:16, :])

                class HB:
                    def __getitem__(self, idx):
                        p, kc, s = idx
                        return (ta[p, kc, s] if kc < 8
                                else tb[p, kc - 8, s])
                return HB()

            def rope_norm_chunk(hblk, w_sb, mslice, csA, csB, out_dst):
                """Project one 128-row chunk, rope+rms-norm it, write bf16."""
                ps = psb.tile([128, 512], f32, tag="big")
                for kc in range(KC):
                    nc.tensor.matmul(ps, w_sb[:, kc, mslice], hblk[:, kc, :],
                                     start=(kc == 0), stop=(kc == KC - 1))
                # single fast read of ps, then everything works off qsb
                qsb = wk.tile([128, 512], bf16, tag="qsb")
                nc.vector.tensor_copy(out=qsb, in_=ps)
                # squared sums per 64-row half -> replicated across the half
                sq_t = wk.tile([128, 512], bf16, tag="sq")
                nc.scalar.activation(out=sq_t, in_=qsb, func=Square)
                sqps = psx.tile([128, 512], f32, tag="aux")
                nc.tensor.matmul(sqps, ones128, sq_t, start=True, stop=True)
                # rstd = exp(-0.5*ln(ms/HD + eps))
                lnm = wk.tile([128, 512], f32, tag="lnm")
                nc.scalar.activation(out=lnm, in_=sqps, func=Ln,
                                     scale=1.0 / HD, bias=eps_t)
                rb = wk.tile([128, 512], f32, tag="rb")
                nc.scalar.activation(out=rb, in_=lnm, func=Exp, scale=-0.5)
                # rope: qa = qsb*csA + (rotP @ qsb)*csB
                rps = psx.tile([128, 512], f32, tag="aux")
                nc.tensor.matmul(rps, rot_sb, qsb, start=True, stop=True)
                a1 = wk.tile([128, 512], f32, tag="a1")
                nc.vector.tensor_mul(a1, qsb, csA)
                a2 = wk.tile([128, 512], f32, tag="a2")
                nc.vector.tensor_mul(a2, rps, csB)
                nc.vector.tensor_add(a1, a1, a2)
                nc.vector.tensor_mul(out_dst, a1, rb)

            def proj_block(sb):
                s0 = sb * 512
                sl = slice(s0, s0 + 512)
                hblk = load_hblk(sb)
                # ---- k (both kv heads in one chunk) ----
                rope_norm_chunk(hblk, wk_sb, slice(0, KD),
                                csA_k[:, sl], csB_k[:, sl], kT_sb[:, sl])
                # ---- v in natural layout: h-block chunk as stationary ----
                for cc in range(4):
                    vps = psx.tile([128, 128], f32, tag="aux")
                    for kc in range(KC):
                        nc.tensor.matmul(vps,
                                         hblk[:, kc, cc * 128:(cc + 1) * 128],
                                         wv_sb[:, kc, :],
                                         start=(kc == 0), stop=(kc == KC - 1))
                    chunk = sb * 4 + cc
                    nc.vector.tensor_copy(out=v_sb[:, chunk, 0, 0:64],
                                          in_=vps[:, 0:64])
                    nc.vector.tensor_copy(out=v_sb[:, chunk, 1, 64:128],
                                          in_=vps[:, 64:128])
                # ---- q (4 chunks; chunk m = heads (m, m+4)) ----
                qT = qT_tile()
                qT_blk[sb] = qT
                for m in range(4):
                    rope_norm_chunk(hblk, wq_sb,
                                    slice(m * 128, (m + 1) * 128),
                                    csA_q[:, sl], csB_q[:, sl], qT[:, m, :])
                # ---- gate: sigmoid(x) = exp(-ln(1+exp(-x))) ----
                g = g_tile()
                g_blk[sb] = g
                for m in range(4):
                    ps = psb.tile([128, 512], f32, tag="big")
                    for kc in range(KC):
                        nc.tensor.matmul(ps, wg_sb[:, kc,
                                                   m * 128:(m + 1) * 128],
                                         hblk[:, kc, :],
                                         start=(kc == 0), stop=(kc == KC - 1))
                    e = wk.tile([128, 512], f32, tag="a1", name="ge")
                    nc.scalar.activation(out=e, in_=ps, func=Exp, scale=-1.0)
                    t = wk.tile([128, 512], f32, tag="a2", name="gt")
                    nc.scalar.activation(out=t, in_=e, func=Ln, bias=1.0)
                    nc.scalar.activation(out=g[:, m, :], in_=t, func=Exp,
                                         scale=-1.0)

            def attn_block(sb):
                """Attention for q block sb (512 queries), all 4 head pairs."""
                s0 = sb * 512
                nkc = 4 * (sb + 1)
                qT = qT_blk[sb]
                g = g_blk[sb]
                og = og_tile()
                og_blk[sb] = og
                for p in range(4):
                    av = psav.tile([128, 2, 512], f32, tag="av")
                    avA = av[:, 0, :]
                    avB = av[:, 1, :]
                    scq = pssc.tile([128, 2, 2, 512], f32, tag="sc")
                    for kc in range(nkc):
                        rr = kc % 2
                        diag_i = kc - 4 * sb
                        trim = 128 * diag_i if diag_i > 0 else 0
                        n = 512 - trim
                        qs = slice(trim, 512)
                        sc = scq[:, rr, :, :]
                        nc.tensor.matmul(
                            sc[:, 0, qs],
                            kT_sb[0:64, kc * 128:(kc + 1) * 128],
                            qT[0:64, p, qs], start=True, stop=True)
                        nc.tensor.matmul(
                            sc[:, 1, qs],
                            kT_sb[64:128, kc * 128:(kc + 1) * 128],
                            qT[64:128, p, qs], start=True, stop=True)
                        probs = aw.tile([128, 2, 512], bf16, tag="probs")
                        nc.scalar.activation(
                            out=probs[:, :, qs], in_=sc[:, :, qs],
                            func=Exp, scale=SCALE)
                        if diag_i >= 0:
                            # causal: keep col j' >= partition p (same mask
                            # for every diagonal block after trimming)
                            nc.vector.tensor_mul(probs[:, 0, qs],
                                                 probs[:, 0, qs],
                                                 cm_sb[:, 0:n])
                            nc.vector.tensor_mul(probs[:, 1, qs],
                                                 probs[:, 1, qs],
                                                 cm_sb[:, 0:n])
                        if debug_dump and p == 0:
                            nc.sync.dma_start(out=dbg_pr[:, sb, kc, :, qs],
                                              in_=probs[:, :, qs])
                        nc.tensor.matmul(
                            av[0:65, 0, qs], v_sb[:, kc, 0, 0:65],
                            probs[:, 0, qs],
                            start=(kc == 0), stop=(kc == nkc - 1))
                        nc.tensor.matmul(
                            av[:, 1, qs], v_sb[:, kc, 1, :],
                            probs[:, 1, qs],
                            start=(kc == 0), stop=(kc == nkc - 1))
                    # normalize + gate. A: dims rows 0:64, denom row 64.
                    #                   B: dims rows 64:128, denom row 0.
                    lnA = ow.tile([128, 512], f32, tag="lnd")
                    nc.scalar.activation(out=lnA[64:65, :], in_=av[64:65, 0, :],
                                         func=Ln)
                    nc.scalar.activation(out=lnA[0:1, :], in_=av[0:1, 1, :],
                                         func=Ln)
                    if debug_dump and p == 0:
                        avd = ow.tile([128, 512], f32, tag="avd", name="avd")
                        nc.vector.tensor_copy(out=avd[64:65, :],
                                              in_=av[64:65, 0, :])
                        nc.vector.tensor_copy(out=avd[0:1, :],
                                              in_=av[0:1, 1, :])
                        nc.sync.dma_start(out=dbg_av[:, sb, :], in_=avd)
                        nc.sync.dma_start(out=dbg_ln[:, sb, :], in_=lnA)
                    rA = ow.tile([128, 512], f32, tag="rd")
                    nc.scalar.activation(out=rA[64:65, :], in_=lnA[64:65, :],
                                         func=Exp, scale=-1.0)
                    nc.scalar.activation(out=rA[0:1, :], in_=lnA[0:1, :],
                                         func=Exp, scale=-1.0)
                    # partition_broadcast only works base0 -> base0; route
                    # cross-partition moves through (gpsimd) DMA instead.
                    rbv = ow.tile([128, 512], f32, tag="lnd", name="rbv")
                    rdup = ow.tile([128, 512], f32, tag="rdup")
                    rbt = ow.tile([64, 512], f32, tag="rdup", name="rbt")
                    nc.sync.dma_start(out=rdup[0:1, :], in_=rA[64:65, :])
                    nc.gpsimd.partition_broadcast(rbv[0:64, :], rdup[0:1, :])
                    nc.gpsimd.partition_broadcast(rbt[0:64, :], rA[0:1, :])
                    nc.sync.dma_start(out=rbv[64:128, :], in_=rbt[0:64, :])
                    if debug_dump and p == 0:
                        nc.sync.dma_start(out=dbg_rb[:, sb, :], in_=rbv)
                        nc.sync.dma_start(out=dbg_rd[:, sb, :], in_=rA)
                    tmpA = ow.tile([128, 512], bf16, tag="lnd", name="ogt")
                    nc.vector.tensor_mul(tmpA[0:64, :], av[0:64, 0, :],
                                         rbv[0:64, :])
                    nc.vector.tensor_mul(tmpA[64:128, :], av[64:128, 1, :],
                                         rbv[64:128, :])
                    nc.vector.tensor_mul(og[0:64, p, :], tmpA[0:64, :],
                                         g[0:64, p, :])
                    nc.vector.tensor_mul(og[64:128, p, :], tmpA[64:128, :],
                                         g[64:128, p, :])

            def oproj_block(sb):
                og = og_blk[sb]
                for m in range(KC):
                    po = psb.tile([128, 512], f32, tag="big")
                    for oc in range(4):
                        nc.tensor.matmul(po, wo_sb[:, oc, m, :],
                                         og[:, oc, :],
                                         start=(oc == 0), stop=(oc == 3))
                    stg = ostg.tile([128, 512], bf16, tag="stg")
                    nc.vector.tensor_copy(out=stg, in_=po)
                    nc.sync.dma_start(
                        out=outT[m * 128:(m + 1) * 128,
                                 sb * 512:(sb + 1) * 512],
                        in_=stg)

            for sb in range(SB):
                proj_block(sb)
                attn_block(sb)
                oproj_block(sb)
                if debug_dump:
                    ssl = slice(sb * 512, (sb + 1) * 512)
                    nc.sync.dma_start(out=dbg_q[:, :, ssl], in_=qT_blk[sb])
                    nc.sync.dma_start(out=dbg_g[:, :, ssl], in_=g_blk[sb])
                    nc.sync.dma_start(out=dbg_og[:, :, ssl], in_=og_blk[sb])
            if debug_dump:
                nc.sync.dma_start(out=dbg_k, in_=kT_sb)
                nc.sync.dma_start(out=dbg_v, in_=v_sb)

    nc.compile()
    return nc


def _host_prep(hidden_states, cos, sin, Wq, Wk, Wv, Wg, Wo, q_norm_w, k_norm_w):
    """Build per-core input maps."""
    half = ROPE // 2  # 16
    # head order within a core: pairs (m, m+4) -> chunk m
    perm = [0, 4, 1, 5, 2, 6, 3, 7]

    def cs_tables(cos_b, sin_b, w):
        # csA/csB [128, S]: row p -> head-local dim d = p % 64
        A = np.empty((128, S), np.float32)
        Bt = np.empty((128, S), np.float32)
        cosT = cos_b.T  # [32, S]
        sinT = sin_b.T
        for blk in (0, 64):
            A[blk + 0:blk + 32] = cosT * w[0:32, None]
            A[blk + 32:blk + 64] = w[32:64, None]
            Bt[blk + 0:blk + 16] = -sinT[0:16] * w[16:32, None]
            Bt[blk + 16:blk + 32] = sinT[16:32] * w[0:16, None]
            Bt[blk + 32:blk + 64] = 0.0
        return A.astype(BF16), Bt.astype(BF16)

    # rotP: out[d] = q[d+16] (d%64 in 0:16), q[d-16] (d%64 in 16:32), else 0
    rotP = np.zeros((128, 128), np.float32)
    for hh in (0, 64):
        for j in range(16):
            rotP[hh + 16 + j, hh + j] = 1.0      # out[hh+j] = in[hh+16+j]
            rotP[hh + j, hh + 16 + j] = 1.0      # out[hh+16+j] = in[hh+j]
    rotP = rotP.astype(BF16)
    cmask = (np.arange(512)[None, :] >= np.arange(128)[:, None]).astype(BF16)

    in_maps = []
    for c in range(NCORES):
        b, g = c // 4, c % 4
        qs = slice(g * QD, (g + 1) * QD)
        ks = slice(g * KD, (g + 1) * KD)
        csA_q, csB_q = cs_tables(cos[b], sin[b], np.asarray(q_norm_w))
        csA_k, csB_k = cs_tables(cos[b], sin[b], np.asarray(k_norm_w))
        Wq_l = Wq[qs].reshape(QH, HD, HID)[perm].reshape(QD, HID)
        Wg_l = Wg[qs].reshape(QH, HD, HID)[perm].reshape(QD, HID)
        Wo_l = Wo[:, qs].reshape(HID, QH, HD)[:, perm].reshape(HID, QD)
        in_maps.append({
            "hT": np.ascontiguousarray(hidden_states[b].T).astype(BF16),
            "wqT": np.ascontiguousarray(Wq_l.T).astype(BF16),
            "wkT": np.ascontiguousarray(Wk[ks].T).astype(BF16),
            "wvT": np.ascontiguousarray(Wv[ks].T).astype(BF16),
            "wgT": np.ascontiguousarray(Wg_l.T).astype(BF16),
            "woT": np.ascontiguousarray(Wo_l.T).astype(BF16),
            "csAq": csA_q, "csBq": csB_q, "csAk": csA_k, "csBk": csB_k,
            "rotP": rotP, "cmsk": cmask,
        })
    return in_maps


def kernel(hidden_states, cos, sin, Wq, Wk, Wv, Wg, Wo, q_norm_w, k_norm_w):
    from concourse import bass_utils

    if "nc" not in _CACHE:
        _CACHE["nc"] = _build_bass()
    nc = _CACHE["nc"]

    in_maps = _host_prep(hidden_states, cos, sin, Wq, Wk, Wv, Wg, Wo,
                         q_norm_w, k_norm_w)

    trace = bool(int(os.environ.get("KERNEL_TRACE", "0")))
    kwargs = {}
    if trace:
        # the agent image's antenv lacks axon_hooks; recreate it from the
        # boot helper so run_bass_kernel_spmd(trace=True) can NTFF-profile
        try:
            import antenv.axon_hooks  # noqa: F401
        except ImportError:
            import types
            sys.path.insert(0, "/root/.axon_site")
            from trn_agent_boot.trn_boot import _ntff_profile_via_ctypes
            hook = _ntff_profile_via_ctypes("/opt/axon/libaxon_pjrt.so")
            mod = types.ModuleType("antenv.axon_hooks")
            mod.get_axon_ntff_profile_hook = lambda: hook
            sys.modules["antenv.axon_hooks"] = mod
        tmpdir = os.environ.get("KERNEL_TRACE_DIR") or None
        kwargs = dict(trace=True, tmpdir=tmpdir)
    res = bass_utils.run_bass_kernel_spmd(nc, in_maps,
                                          core_ids=list(range(NCORES)),
                                          **kwargs)
    if trace and res.exec_time_ns is not None:
        print(f"HW exec time: {res.exec_time_ns} ns")
        _CACHE["exec_time_ns"] = res.exec_time_ns

    out = np.zeros((B, S, HID), np.float32)
    for c in range(NCORES):
        b = c // 4
        out[b] += res.results[c]["outT"].T.astype(np.float32)
    return out


if __name__ == "__main__":
    rng = np.random.default_rng(0)
    hs = rng.standard_normal((B, S, HID), dtype=np.float32)
    cos = rng.random((B, S, ROPE), dtype=np.float32)
    sin = rng.random((B, S, ROPE), dtype=np.float32)
    out = kernel(hidden_states=hs, cos=cos, sin=sin,
                 Wq=rng.standard_normal((NH * HD, HID), dtype=np.float32) * 0.02,
                 Wk=rng.standard_normal((NKV * HD, HID), dtype=np.float32) * 0.02,
                 Wv=rng.standard_normal((NKV * HD, HID), dtype=np.float32) * 0.02,
                 Wg=rng.standard_normal((NH * HD, HID), dtype=np.float32) * 0.02,
                 Wo=rng.standard_normal((HID, NH * HD), dtype=np.float32) * 0.02,
                 q_norm_w=np.ones(HD, np.float32),
                 k_norm_w=np.ones(HD, np.float32))
    print(out.shape, out.dtype)
